# revision 33
# baseline (speedup 1.0000x reference)
"""MALA attention (linear attention w/ 2D RoPE + magnitude term) on 8 trn2 cores.

Sharding: core i handles batch b = i//2, sequence rows (i%2)*2048..+2048.
Cross-core data (kv = k_rope^T v, k_sum, v_sum -- all sums over the full
sequence) is combined with a pairwise AllReduce (~400KB). Everything else is
local. bf16 operands everywhere with fp32 PSUM accumulation.

Math (per batch b, head h, reference semantics):
  q = elu(query @ Wq.T + bq) + 1      (same k with Wk/bk; v plain)
  z = (q . mean_n(k)) * d^-0.5
  q, k <- rope(q), rope(k)
  kv = k^T v * (d^-0.5 / N)
  res = (q @ kv) * (1 + 1/(z+1e-6)) - z * mean_n(v)
  out = res @ Wo.T + bo

Device-side formulation:
  elu(x)+1 = max(x + 1, min(exp(x), 1))    [exact: exp(min(x,0)) = min(exp x,1)]
  bias+1 folded into the projection via a rank-1 (K=1) matmul.
  g = 1 + 1/(z+eps) applied after q@kv (g is constant along the head's d/e
  dims, so it commutes with the per-head contraction).
  -z*v_mean term folded into o_proj as an extra matmul (-W~)^T @ z where
  W~[c,h] = Wo[:, head h] @ v_mean_h.
  The within-head d-index is permuted (evens first) on Wq/Wk columns + trig
  tables so rotate_every_two becomes a 32-column block swap.

Scheduling notes (v2):
  - All weight/const DMAs are issued up front; q-side consts go on the
    gpsimd DGE queue so they don't delay phase-1 x-chunk loads on sync.
  - All collective-dependent work (bounce reads, kblock/vbneg/kv2sb
    assembly) lives on the gpsimd queue/engine so its wait on the
    AllReduce can't head-of-line-block sync DMAs or DVE work.
  - Post-collective small tensors are read from the bounce buffer with 6
    strided DMAs + constant-mask multiplies instead of 36 tiny DMAs.
  - res = (q@kv) * g is one DVE op reading two PSUMs (no scalar copy).
  - Output DMAs issue from gpsimd (idle engine) to keep sync free.
"""

import os
import tempfile

import numpy as np
import ml_dtypes

NUM_HEADS = 12
B, N, C = 4, 4096, 768
D = 64
NCORES = 8
NLOC = N // 2          # rows per core
SCALE = D ** -0.5
BF16 = ml_dtypes.bfloat16

_CACHE = {}
LAST_RESULTS = [None]  # test.py reads profiling info from here


# --------------------------------------------------------------------------
# host-side helpers
# --------------------------------------------------------------------------

def _perm64():
    # evens first, then odds (within each head's 64 dims)
    return np.concatenate([np.arange(0, 64, 2), np.arange(1, 64, 2)])


def _trig_tables():
    """c32/s32: [N, 32] fp32, value of cos/sin at original dim 2i (== 2i+1)."""
    H = W = 64
    angle = 1.0 / (10000.0 ** np.linspace(0.0, 1.0, D // 4))
    angle = np.repeat(angle, 2)                          # [32]
    ih = np.arange(H, dtype=np.float64)[:, None] * angle[None, :]   # [H, 32]
    iw = np.arange(W, dtype=np.float64)[:, None] * angle[None, :]
    # full table col j<32 from sin_h (depends on row r = n//W), col 32+j from
    # sin_w (depends on col c = n%W). Original pair (2i, 2i+1):
    #   2i < 32  -> table col 2i   in h-block, base angle idx 2i
    #   2i >= 32 -> in w-block, idx 2i-32
    sin_h, cos_h = np.sin(ih), np.cos(ih)
    sin_w, cos_w = np.sin(iw), np.cos(iw)
    r = np.arange(N) // W
    c = np.arange(N) % W
    s_full = np.concatenate([sin_h[r], sin_w[c]], axis=1)   # [N, 64]
    c_full = np.concatenate([cos_h[r], cos_w[c]], axis=1)
    c32 = c_full[:, 0::2].astype(np.float32)
    s32 = s_full[:, 0::2].astype(np.float32)
    return c32, s32


def _head_masks():
    """maskS/maskN: [128, 6, 12] bf16; nonzero at (p, s, h) iff
    h == 2s + (p>=64), value SFAC resp. -1/N."""
    sfac = SCALE / N
    mS = np.zeros((128, 6, NUM_HEADS), dtype=np.float32)
    mN = np.zeros((128, 6, NUM_HEADS), dtype=np.float32)
    for p in range(128):
        for s in range(6):
            h = 2 * s + (1 if p >= 64 else 0)
            mS[p, s, h] = sfac
            mN[p, s, h] = -1.0 / N
    return mS.astype(BF16), mN.astype(BF16)


def _build_host_inputs(query, key, value, Wq, bq, Wk, bk, Wv, bv, Wo, bo):
    p64 = _perm64()
    perm = (np.arange(NUM_HEADS)[:, None] * 64 + p64[None, :]).reshape(-1)

    wq = np.ascontiguousarray(Wq.T[:, perm]).astype(BF16)
    wk = np.ascontiguousarray(Wk.T[:, perm]).astype(BF16)
    wv = np.ascontiguousarray(Wv.T).astype(BF16)
    wo = np.ascontiguousarray(Wo.T).astype(BF16)
    bqp1 = (bq[perm] + 1.0).astype(BF16)
    bkp1 = (bk[perm] + 1.0).astype(BF16)
    bvp = bv.astype(BF16)
    bof = bo.astype(np.float32)

    c32, s32 = _trig_tables()
    halves = []
    for hi in range(2):
        sl = slice(hi * NLOC, (hi + 1) * NLOC)
        ck = np.concatenate([c32[sl], c32[sl]], axis=1).astype(BF16)    # [NLOC, 64]
        s2k = np.concatenate([-s32[sl], s32[sl]], axis=1).astype(BF16)
        cq = np.tile(c32[sl].T, (4, 1)).astype(BF16)                    # [128, NLOC]
        s2q = np.tile(np.concatenate([-s32[sl].T, s32[sl].T], 0), (2, 1)).astype(BF16)
        halves.append((ck, s2k, cq, s2q))

    # one-hot G-broadcast lhsT: eblk[c][h, p] = 1 iff head h owns partition p
    # of chunk c (heads 2c: p<64, 2c+1: p>=64)
    eblk = np.zeros((6, NUM_HEADS, 128), dtype=BF16)
    for cc in range(6):
        eblk[cc, 2 * cc, :64] = 1.0
        eblk[cc, 2 * cc + 1, 64:] = 1.0

    maskS, maskN = _head_masks()

    in_maps = []
    for core in range(NCORES):
        b = core // 2
        hi = core % 2
        sl = slice(hi * NLOC, (hi + 1) * NLOC)
        ck, s2k, cq, s2q = halves[hi]
        in_maps.append({
            "xq": np.ascontiguousarray(query[b, sl].T).astype(BF16),
            "xk": np.ascontiguousarray(key[b, sl].T).astype(BF16),
            "xv": np.ascontiguousarray(value[b, sl].T).astype(BF16),
            "wq": wq, "wk": wk, "wv": wv, "wo": wo,
            "bqp1": bqp1, "bkp1": bkp1, "bv": bvp, "bo": bof,
            "cos_k": ck, "s2_k": s2k, "cos_q": cq, "s2_q": s2q,
            "eblk": eblk, "maskS": maskS, "maskN": maskN,
        })
    return in_maps


# --------------------------------------------------------------------------
# device kernel
# --------------------------------------------------------------------------

def _build_nc():
    import concourse.bacc as bacc
    import concourse.mybir as mybir
    import concourse.tile as tile

    fp32 = mybir.dt.float32
    bf16 = mybir.dt.bfloat16
    AF = mybir.ActivationFunctionType
    OP = mybir.AluOpType

    nc = bacc.Bacc("TRN2", target_bir_lowering=False, debug=False,
                   num_devices=NCORES)

    def din(name, shape, dt=bf16):
        return nc.dram_tensor(name, shape, dt, kind="ExternalInput").ap()

    xq = din("xq", [C, NLOC])
    xk = din("xk", [C, NLOC])
    xv = din("xv", [C, NLOC])
    wq_d = din("wq", [C, C])
    wk_d = din("wk", [C, C])
    wv_d = din("wv", [C, C])
    wo_d = din("wo", [C, C])
    bqp1_d = din("bqp1", [C])
    bkp1_d = din("bkp1", [C])
    bv_d = din("bv", [C])
    bo_d = din("bo", [C], fp32)
    cos_k_d = din("cos_k", [NLOC, 64])
    s2_k_d = din("s2_k", [NLOC, 64])
    cos_q_d = din("cos_q", [128, NLOC])
    s2_q_d = din("s2_q", [128, NLOC])
    eblk_d = din("eblk", [6, NUM_HEADS, 128])
    maskS_d = din("maskS", [128, 6, NUM_HEADS])
    maskN_d = din("maskN", [128, 6, NUM_HEADS])
    outT = nc.dram_tensor("outT", [C, NLOC], fp32, kind="ExternalOutput").ap()

    SFAC = SCALE / N       # kv and z scale
    NF = NLOC // 512       # 4 n-slices of 512
    NK = NLOC // 128       # 16 chunks of 128 for k/v phase

    with tile.TileContext(nc) as tc:
        from contextlib import ExitStack
        with ExitStack() as ctx:
            consts = ctx.enter_context(tc.tile_pool(name="consts", bufs=1))
            resid = ctx.enter_context(tc.tile_pool(name="resid", bufs=1))
            xin = ctx.enter_context(tc.tile_pool(name="xin", bufs=2))
            work = ctx.enter_context(tc.tile_pool(name="work", bufs=2))
            single = ctx.enter_context(tc.tile_pool(name="single", bufs=1))
            big = ctx.enter_context(tc.tile_pool(name="big", bufs=2))
            dram = ctx.enter_context(tc.tile_pool(name="dram", bufs=1, space="DRAM"))
            ph1 = ExitStack()
            pps = ph1.enter_context(tc.tile_pool(name="pps", bufs=5, space="PSUM"))
            kvp = ph1.enter_context(tc.tile_pool(name="kvp", bufs=1, space="PSUM"))

            # ---- phase-1 k-side on the sync queue (wk split so chunk-0
            # matmuls can start after half the weight transfer) ----
            bkp1_s = consts.tile([1, C], bf16, tag="bkp1")
            nc.sync.dma_start(bkp1_s[:], bkp1_d[None, :])
            bv_s = consts.tile([1, C], bf16, tag="bv")
            nc.sync.dma_start(bv_s[:], bv_d[None, :])
            wk_v = wk_d.rearrange("(s p) o -> p s o", p=128)
            wk_s = consts.tile([128, 6, C], bf16, tag="wk")
            nc.sync.dma_start(wk_s[:, 0:3, :], wk_v[:, 0:3, :])
            # (second wk half + x chunks interleave into the chunk loop)
            wv_s = consts.tile([128, 6, C], bf16, tag="wv")

            def xq_dma(nq, eng=None):
                t = xin.tile([128, 6, 512], bf16, tag="xq_t", name=f"xq{nq}")
                (eng or nc.sync).dma_start(
                    t[:], xq[:, nq * 512:(nq + 1) * 512]
                    .rearrange("(s p) n -> p s n", p=128))
                return t
            xq_tiles = {}

            # ---- v-weights + trig early on the gpsimd queue (second DMA
            # ring; xv chunk stream also rides here) ----
            wv_v = wv_d.rearrange("(s p) o -> p s o", p=128)
            nc.gpsimd.dma_start(wv_s[:, 0:3, :], wv_v[:, 0:3, :])
            cos_k_s = consts.tile([128, NK, 64], bf16, tag="cos_k")
            s2_k_s = consts.tile([128, NK, 64], bf16, tag="s2_k")
            # declared here, transferred after the chunk loop (gpsimd queue)
            wq_s = consts.tile([128, 6, C], bf16, tag="wq")
            wo_s = consts.tile([128, 6, C], bf16, tag="wo")
            bqp1_s = consts.tile([1, C], bf16, tag="bqp1")
            bo_s = consts.tile([128, 6], fp32, tag="bo")
            cos_q_s = consts.tile([128, NLOC], bf16, tag="cos_q")
            s2_q_s = consts.tile([128, NLOC], bf16, tag="s2_q")
            eblk_s = consts.tile([NUM_HEADS, 6, 128], bf16, tag="eblk")
            maskS_s = consts.tile([128, 6, NUM_HEADS], bf16, tag="maskS")
            maskN_s = consts.tile([128, 6, NUM_HEADS], bf16, tag="maskN")

            ones_row = consts.tile([1, 512], bf16, tag="ones_row")
            nc.vector.memset(ones_row[:], 1.0)
            negone = consts.tile([128, 1], fp32, tag="negone")
            nc.vector.memset(negone[:], -1.0)
            zero_col = consts.tile([1, 128], bf16, tag="zero_col")
            nc.vector.memset(zero_col[:], 0.0)

            # phase-2b staging (memset early on vector; filled post-collective)
            kv2f = single.tile([128, 6, 128], bf16, tag="kv2f")
            nc.vector.memset(kv2f[:], 0.0)
            kscat = single.tile([128, 2, 6], bf16, tag="kscat")
            nc.vector.memset(kscat[:], 0.0)
            vscat = single.tile([128, 2, 6], bf16, tag="vscat")
            nc.vector.memset(vscat[:], 0.0)

            # ---- persistent tiles ----
            qpre = big.tile([128, 6, NLOC], bf16, tag="qbig", name="qpre")
            qrope = resid.tile([128, 6, NLOC], bf16, tag="qrope")
            res = big.tile([128, 6, NLOC], bf16, tag="qbig", name="res")
            zb = resid.tile([NUM_HEADS, NLOC], bf16, tag="zb")
            kv2sb = resid.tile([128, 6, 128], bf16, tag="kv2sb")
            kblock = resid.tile([128, 6, NUM_HEADS], bf16, tag="kblock")
            wneg = resid.tile([NUM_HEADS, C], bf16, tag="wneg")

            # kv psums: 3 banks, persist through phase 1.
            # head h accumulates at [0:65, (h%4)*128 : +128] of tile h//4.
            kvps = [kvp.tile([128, 512], fp32, tag=f"kvps{t}", name=f"kvps{t}")
                    for t in range(3)]
            # start=True clears the whole bank's has_written bits, so packing 4
            # heads' accumulation groups per bank needs a single bank-wide
            # zero-write group opener; all kv matmuls then accumulate.
            for t in range(3):
                nc.tensor.matmul(kvps[t][0:65, :], zero_col[:, 0:65], ones_row[:],
                                 start=True, stop=False, skip_group_check=True)

            # ================= phase 1: k/v proj, elu, rope, kv =================
            for j in range(NK):
                xk_t = xin.tile([128, 6, 128], bf16, tag="xk_t")
                nc.sync.dma_start(
                    xk_t[:], xk[:, j * 128:(j + 1) * 128]
                    .rearrange("(s p) n -> p s n", p=128))
                xv_t = xin.tile([128, 6, 128], bf16, tag="xv_t")
                nc.gpsimd.dma_start(
                    xv_t[:], xv[:, j * 128:(j + 1) * 128]
                    .rearrange("(s p) n -> p s n", p=128))
                if j == 0:
                    nc.sync.dma_start(wk_s[:, 3:6, :], wk_v[:, 3:6, :])
                    nc.gpsimd.dma_start(wv_s[:, 3:6, :], wv_v[:, 3:6, :])
                    nc.gpsimd.dma_start(
                        cos_k_s[:], cos_k_d.rearrange("(j p) d -> p j d", p=128))
                    nc.gpsimd.dma_start(
                        s2_k_s[:], s2_k_d.rearrange("(j p) d -> p j d", p=128))
                elif j == 8:
                    xq_tiles[0] = xq_dma(0, nc.gpsimd)
                elif j == 12:
                    xq_tiles[1] = xq_dma(1, nc.gpsimd)

                vk = work.tile([128, NUM_HEADS, 128], bf16, tag="vk")
                e_t = work.tile([128, C], bf16, tag="e_t")
                kra = work.tile([128, NUM_HEADS, 66], bf16, tag="kra")
                nc.vector.memset(kra[:, :, 64:65], 1.0)

                # k projection: s-outer so each stationary x-slice is one
                # LDWEIGHTS feeding both o-halves
                psk = [pps.tile([128, 384], fp32, tag="pp384", name=f"psk{half}")
                       for half in range(2)]
                for half in range(2):
                    nc.tensor.matmul(psk[half][:], ones_row[:, 0:128],
                                     bkp1_s[:, half * 384:(half + 1) * 384],
                                     start=True, stop=False)
                for s in range(6):
                    for half in range(2):
                        nc.tensor.matmul(psk[half][:], xk_t[:, s, :],
                                         wk_s[:, s, half * 384:(half + 1) * 384],
                                         start=False, stop=(s == 5))
                for half in range(2):
                    osl = slice(half * 384, (half + 1) * 384)
                    hsl = slice(half * 6, (half + 1) * 6)
                    nc.scalar.activation(e_t[:, osl], psk[half][:], AF.Exp,
                                         bias=negone[:])
                    nc.vector.scalar_tensor_tensor(
                        vk[:, hsl, 64:128],
                        e_t[:, osl].rearrange("p (h e) -> p h e", e=64),
                        1.0, psk[half][:].rearrange("p (h e) -> p h e", e=64),
                        OP.min, OP.max)

                # v projection
                psv = [pps.tile([128, 384], fp32, tag="pp384", name=f"psv{half}")
                       for half in range(2)]
                for half in range(2):
                    nc.tensor.matmul(psv[half][:], ones_row[:, 0:128],
                                     bv_s[:, half * 384:(half + 1) * 384],
                                     start=True, stop=False)
                for s in range(6):
                    for half in range(2):
                        nc.tensor.matmul(psv[half][:], xv_t[:, s, :],
                                         wv_s[:, s, half * 384:(half + 1) * 384],
                                         start=False, stop=(s == 5))
                for half in range(2):
                    hsl = slice(half * 6, (half + 1) * 6)
                    nc.scalar.activation(
                        vk[:, hsl, 0:64],
                        psv[half][:].rearrange("p (h e) -> p h e", e=64),
                        AF.Copy)

                # rope on k_pre -> kra[:, :, 0:64]
                kpre_v = vk[:, :, 64:128]
                cosj = cos_k_s[:, j, None, :].to_broadcast([128, NUM_HEADS, 64])
                nc.vector.tensor_tensor(kra[:, :, 0:64], kpre_v, cosj, OP.mult)
                tmpb = work.tile([128, NUM_HEADS, 64], bf16, tag="tmpb")
                s2t = s2_k_s[:, j, None, 0:32].to_broadcast([128, NUM_HEADS, 32])
                s2b = s2_k_s[:, j, None, 32:64].to_broadcast([128, NUM_HEADS, 32])
                nc.vector.tensor_tensor(tmpb[:, :, 0:32], vk[:, :, 96:128], s2t, OP.mult)
                nc.vector.tensor_tensor(tmpb[:, :, 32:64], vk[:, :, 64:96], s2b, OP.mult)
                nc.vector.tensor_tensor(kra[:, :, 0:64], kra[:, :, 0:64], tmpb[:],
                                        OP.add)

                # kv accumulation: [k_rope | 1]^T @ [v | k_pre] per head
                for h in range(NUM_HEADS):
                    nc.tensor.matmul(
                        kvps[h // 4][0:65, (h % 4) * 128:(h % 4) * 128 + 128],
                        kra[:, h, 0:65], vk[:, h, :],
                        start=False, stop=(j == NK - 1), skip_group_check=True)

            # ---- q/o-side constants (gpsimd ring, after the xv stream) ----
            nc.gpsimd.dma_start(wq_s[:], wq_d.rearrange("(s p) o -> p s o", p=128))
            nc.gpsimd.dma_start(wo_s[:], wo_d.rearrange("(s p) o -> p s o", p=128))
            nc.gpsimd.dma_start(bqp1_s[:], bqp1_d[None, :])
            nc.gpsimd.dma_start(bo_s[:], bo_d.rearrange("(s p) -> p s", p=128))
            nc.gpsimd.dma_start(cos_q_s[:], cos_q_d[:])
            nc.gpsimd.dma_start(s2_q_s[:], s2_q_d[:])
            nc.gpsimd.dma_start(eblk_s[:], eblk_d.rearrange("c h p -> h c p"))
            nc.gpsimd.dma_start(maskS_s[:], maskS_d[:])
            nc.gpsimd.dma_start(maskN_s[:], maskN_d[:])

            # ================= collective: kv + sums (bf16 over the wire) ====
            kvsb = single.tile([65, 3, 512], bf16, tag="kvx", name="kvsb")
            for t in range(3):
                nc.scalar.activation(kvsb[:, t, :], kvps[t][0:65, :], AF.Copy)
            # q-proj gate: ones_g == ones_row exactly, but depends on kvsb so
            # gated q-proj slices can't start before the collective window.
            gate_f = single.tile([1, 1], fp32, tag="gate_f")
            nc.scalar.activation(gate_f[:], kvsb[0:1, 0, 0:1], AF.Copy)
            gate_0 = single.tile([1, 1], bf16, tag="gate_0")
            nc.vector.tensor_scalar_mul(gate_0[:], gate_f[:], 0.0)
            ones_g = single.tile([1, 512], bf16, tag="ones_g")
            nc.vector.tensor_tensor(ones_g[:], ones_row[:],
                                    gate_0[0:1, 0:1].to_broadcast([1, 512]),
                                    OP.add)
            bounce_in = dram.tile([65, 3 * 512], bf16, tag="b_in")
            bounce_out = dram.tile([65, 3 * 512], bf16, tag="b_out")
            nc.sync.dma_start(bounce_in[:], kvsb.rearrange("p t f -> p (t f)"))
            for nq in range(2, NF):
                xq_tiles[nq] = xq_dma(nq)
            nc.gpsimd.collective_compute(
                "AllReduce", OP.add,
                replica_groups=[[0, 1], [2, 3], [4, 5], [6, 7]],
                ins=[bounce_in.opt()], outs=[bounce_out.opt()])

            # ====== phase 2b reads: bounce -> SBUF, split across 3 queues ====
            # bounce col layout: head h spans cols [h*128, h*128+128): v-part
            # at +0..64, k-part at +64..128; partition 64 holds the n-sums.
            # cc-major and r-major views of the 1536 cols:
            bv_cc = bounce_out.rearrange("p (cc r) -> p cc r", r=256)
            bv_r = bounce_out.rearrange("p (cc r) -> p r cc", r=256)

            # kv blocks: even heads -> lower-left quads, odd -> upper-right
            # (gpsimd reads here; scalar/sync reads are emitted after the
            # phase-2a loop so their collective-wait can't block its streams)
            # post-collective gate: releases the last q-proj chunks exactly
            # when the AllReduce lands, covering the assembly latency
            gate2_sb = single.tile([1, 1], bf16, tag="gate2_sb")
            nc.gpsimd.dma_start(gate2_sb[:], bounce_out[0:1, 0:1])
            nc.gpsimd.dma_start(kv2f[0:64, :, 0:64], bv_cc[0:64, :, 0:64])
            nc.gpsimd.dma_start(kscat[0:64, 0, :], bv_r[64, 64:128, :])

            # ================= phase 2a: q proj, elu, rope =================
            # Split so the vector queue never head-of-line blocks: projections
            # for nq 0/1 and their ropes first, then the post-collective
            # assembly, then the gated nq 2/3 (whose evacuations may wait on
            # the collective-window matmuls) last.
            def q_proj(nq):
                nsl = slice(nq * 512, (nq + 1) * 512)
                xq_t = xq_tiles[nq]
                for oc in range(6):
                    if nq < 3:
                        ones_nq = ones_row
                    elif oc < 4:
                        ones_nq = ones_g
                    else:
                        ones_nq = ones_g2
                    psq = pps.tile([128, 512], fp32, tag="pp384", name="psq")
                    nc.tensor.matmul(psq[:], bqp1_s[:, oc * 128:(oc + 1) * 128],
                                     ones_nq[:], start=True, stop=False)
                    for s in range(6):
                        nc.tensor.matmul(psq[:], wq_s[:, s, oc * 128:(oc + 1) * 128],
                                         xq_t[:, s, :], start=False, stop=(s == 5))
                    e_q = work.tile([128, 512], bf16, tag="e_q")
                    nc.scalar.activation(e_q[:], psq[:], AF.Exp, bias=negone[:])
                    nc.vector.scalar_tensor_tensor(
                        qpre[:, oc, nsl], e_q[:], 1.0, psq[:], OP.min, OP.max)

            def q_rope(nq):
                nsl = slice(nq * 512, (nq + 1) * 512)
                qsw = work.tile([128, 6, 512], bf16, tag="qsw")
                for g4 in range(4):
                    sp = (g4 ^ 1) * 32
                    nc.sync.dma_start(qsw[g4 * 32:(g4 + 1) * 32, :, :],
                                      qpre[sp:sp + 32, :, nsl])
                for oc in range(6):
                    nc.vector.tensor_tensor(qrope[:, oc, nsl], qpre[:, oc, nsl],
                                            cos_q_s[:, nsl], OP.mult)
                    tmpq = work.tile([128, 512], bf16, tag="tmpq")
                    nc.vector.tensor_tensor(tmpq[:], qsw[:, oc, :], s2_q_s[:, nsl],
                                            OP.mult)
                    nc.vector.tensor_tensor(qrope[:, oc, nsl], qrope[:, oc, nsl],
                                            tmpq[:], OP.add)

            q_proj(0)
            q_proj(1)
            q_rope(0)
            q_rope(1)
            q_proj(2)

            # remaining bounce reads (sync queue is clear of waits by now)
            nc.sync.dma_start(kv2f[64:128, :, 64:128], bv_cc[0:64, :, 128:192])
            nc.sync.dma_start(kscat[64:128, 1, :], bv_r[64, 192:256, :])
            nc.sync.dma_start(vscat[0:64, 0, :], bv_r[64, 0:64, :])
            nc.sync.dma_start(vscat[64:128, 1, :], bv_r[64, 128:192, :])

            # kv2sb scale on DVE (between ropes; waits only on the kv2f reads)
            gate2_0 = single.tile([1, 1], bf16, tag="gate2_0")
            nc.vector.tensor_scalar_mul(gate2_0[:], gate2_sb[:], 0.0)
            ones_g2 = single.tile([1, 512], bf16, tag="ones_g2")
            nc.vector.tensor_tensor(ones_g2[:], ones_row[:],
                                    gate2_0[0:1, 0:1].to_broadcast([1, 512]),
                                    OP.add)
            nc.vector.tensor_scalar_mul(kv2sb[:], kv2f[:], SFAC)
            # kblock/vbneg mask-multiplies on gpsimd (idle post-collective)
            # [128, 12] head-ordered views as [128, hh, par] (h = 2*hh + par)
            kscat_v = kscat.rearrange("p a b -> p b a")
            vscat_v = vscat.rearrange("p a b -> p b a")
            vbneg = single.tile([128, 6, NUM_HEADS], bf16, tag="vbneg")
            for s in range(6):
                nc.vector.tensor_tensor(
                    kblock[:, s, :].rearrange("p (b a) -> p b a", a=2),
                    kscat_v,
                    maskS_s[:, s, :].rearrange("p (b a) -> p b a", a=2),
                    OP.mult)
                nc.vector.tensor_tensor(
                    vbneg[:, s, :].rearrange("p (b a) -> p b a", a=2),
                    vscat_v,
                    maskN_s[:, s, :].rearrange("p (b a) -> p b a", a=2),
                    OP.mult)

            q_proj(3)

            ph1.close()
            pz = ctx.enter_context(tc.tile_pool(name="pz", bufs=1, space="PSUM"))
            pg = ctx.enter_context(tc.tile_pool(name="pg", bufs=2, space="PSUM"))
            pa = ctx.enter_context(tc.tile_pool(name="pa", bufs=2, space="PSUM"))
            po = ctx.enter_context(tc.tile_pool(name="po", bufs=3, space="PSUM"))

            # wneg = -(W~)^T : [12, 768]  (PE work; emitted after phase 2a so
            # its wait on the collective can't block q-proj matmuls)
            for half in range(2):
                osl = slice(half * 384, (half + 1) * 384)
                psw = pg.tile([128, 512], fp32, tag="psg", name="psw")[0:NUM_HEADS, 0:384]
                for s in range(6):
                    nc.tensor.matmul(psw[:], vbneg[:, s, :], wo_s[:, s, osl],
                                     start=(s == 0), stop=(s == 5))
                nc.scalar.activation(wneg[:, osl], psw[:], AF.Copy)

            # == phase 2c+3+4: per-n-slice chain z -> g -> q@kv -> o_proj
            def chain(nq):
                nsl = slice(nq * 512, (nq + 1) * 512)
                psz = pz.tile([128, 512], fp32, tag="psz", name="psz")[0:NUM_HEADS, :]
                for s in range(6):
                    nc.tensor.matmul(psz[:], kblock[:, s, :], qpre[:, s, nsl],
                                     start=(s == 0), stop=(s == 5))
                gf = work.tile([NUM_HEADS, 512], fp32, tag="gf")
                nc.vector.reciprocal_approx_fast(gf[:], psz[:])  # z >= 7, eps moot
                gb = work.tile([NUM_HEADS, 512], bf16, tag="gb")
                nc.vector.tensor_scalar_add(gb[:], gf[:], 1.0)
                nc.vector.tensor_copy(zb[:, nsl], psz[:])
                for cc in range(6):
                    psg = pg.tile([128, 512], fp32, tag="psg", name="psg")
                    nc.tensor.matmul(psg[:], eblk_s[:, cc, :], gb[:],
                                     start=True, stop=True)
                    gx = work.tile([128, 512], bf16, tag="gx")
                    nc.scalar.activation(gx[:], psg[:], AF.Copy)
                    psa = pa.tile([128, 512], fp32, tag="psa", name="psa")
                    nc.tensor.matmul(psa[:], kv2sb[:, cc, :], qrope[:, cc, nsl],
                                     start=True, stop=True)
                    # res = (q@kv) * (1 + 1/z); psa is independent of the
                    # g-chain so the PE never waits on the reciprocal.
                    nc.vector.tensor_tensor(res[:, cc, nsl], psa[:], gx[:],
                                            OP.mult)
                # o_proj for this n-slice (fills PE while next chain's DVE runs)
                for c2 in range(6):
                    c2sl = slice(c2 * 128, (c2 + 1) * 128)
                    pso = po.tile([128, 512], fp32, tag="pso", name="pso")
                    nc.tensor.matmul(pso[:], wneg[:, c2sl], zb[:, nsl],
                                     start=True, stop=False)
                    for s in range(6):
                        nc.tensor.matmul(pso[:], wo_s[:, s, c2sl], res[:, s, nsl],
                                         start=False, stop=(s == 5))
                    osb = work.tile([128, 512], fp32, tag="osb")
                    if c2 % 2 == 0:
                        nc.scalar.activation(osb[:], pso[:], AF.Identity,
                                             bias=bo_s[:, c2:c2 + 1])
                    else:
                        nc.vector.tensor_tensor(
                            osb[:], pso[:],
                            bo_s[:, c2:c2 + 1].to_broadcast([128, 512]),
                            OP.add)
                    (nc.gpsimd if c2 % 2 == 0 else nc.sync).dma_start(
                        outT[c2sl, nsl], osb[:])

            chain(0)
            q_rope(2)
            q_rope(3)
            chain(1)
            chain(2)
            chain(3)

    nc.compile()
    return nc


def _get_nc():
    if "nc" not in _CACHE:
        _CACHE["nc"] = _build_nc()
    return _CACHE["nc"]


# --------------------------------------------------------------------------
# entry point
# --------------------------------------------------------------------------

def kernel(query, key, value, Wq, bq, Wk, bk, Wv, bv, Wo, bo, H, W):
    from concourse.bass_utils import run_bass_kernel_spmd

    assert int(H) == 64 and int(W) == 64
    query = np.asarray(query, np.float32)
    key = np.asarray(key, np.float32)
    value = np.asarray(value, np.float32)
    in_maps = _build_host_inputs(
        query, key, value,
        np.asarray(Wq, np.float32), np.asarray(bq, np.float32),
        np.asarray(Wk, np.float32), np.asarray(bk, np.float32),
        np.asarray(Wv, np.float32), np.asarray(bv, np.float32),
        np.asarray(Wo, np.float32), np.asarray(bo, np.float32))

    nc = _get_nc()
    kwargs = {}
    if os.environ.get("KERNEL_TRACE") == "1":
        kwargs = dict(trace=True, tmpdir=tempfile.mkdtemp(prefix="malat_"))
    r = run_bass_kernel_spmd(nc, in_maps, core_ids=list(range(NCORES)), **kwargs)
    LAST_RESULTS[0] = r

    out = np.empty((B, N, C), np.float32)
    for core in range(NCORES):
        b = core // 2
        sl = slice((core % 2) * NLOC, (core % 2 + 1) * NLOC)
        out[b, sl, :] = r.results[core]["outT"].T
    return out


# revision 34
# speedup vs baseline: 1.0887x; 1.0887x over previous
"""MALA attention (linear attention w/ 2D RoPE + magnitude term) on 8 trn2 cores.

Sharding: core i handles batch b = i//2, sequence rows (i%2)*2048..+2048.
Cross-core data (kv = k_rope^T v, k_sum, v_sum -- all sums over the full
sequence) is combined with a pairwise AllReduce (~400KB). Everything else is
local. bf16 operands everywhere with fp32 PSUM accumulation.

Math (per batch b, head h, reference semantics):
  q = elu(query @ Wq.T + bq) + 1      (same k with Wk/bk; v plain)
  z = (q . mean_n(k)) * d^-0.5
  q, k <- rope(q), rope(k)
  kv = k^T v * (d^-0.5 / N)
  res = (q @ kv) * (1 + 1/(z+1e-6)) - z * mean_n(v)
  out = res @ Wo.T + bo

Device-side formulation:
  elu(x)+1 = max(x + 1, min(exp(x), 1))    [exact: exp(min(x,0)) = min(exp x,1)]
  bias+1 folded into the projection via a rank-1 (K=1) matmul.
  g = 1 + 1/(z+eps) applied after q@kv (g is constant along the head's d/e
  dims, so it commutes with the per-head contraction).
  -z*v_mean term folded into o_proj as an extra matmul (-W~)^T @ z where
  W~[c,h] = Wo[:, head h] @ v_mean_h.
  The within-head d-index is permuted (evens first) on Wq/Wk columns + trig
  tables so rotate_every_two becomes a 32-column block swap.

Scheduling notes (v2):
  - All weight/const DMAs are issued up front; q-side consts go on the
    gpsimd DGE queue so they don't delay phase-1 x-chunk loads on sync.
  - All collective-dependent work (bounce reads, kblock/vbneg/kv2sb
    assembly) lives on the gpsimd queue/engine so its wait on the
    AllReduce can't head-of-line-block sync DMAs or DVE work.
  - Post-collective small tensors are read from the bounce buffer with 6
    strided DMAs + constant-mask multiplies instead of 36 tiny DMAs.
  - res = (q@kv) * g is one DVE op reading two PSUMs (no scalar copy).
  - Output DMAs issue from gpsimd (idle engine) to keep sync free.
"""

import os
import tempfile

import numpy as np
import ml_dtypes

NUM_HEADS = 12
B, N, C = 4, 4096, 768
D = 64
NCORES = 8
NLOC = N // 2          # rows per core
SCALE = D ** -0.5
BF16 = ml_dtypes.bfloat16

_CACHE = {}
LAST_RESULTS = [None]  # test.py reads profiling info from here


# --------------------------------------------------------------------------
# host-side helpers
# --------------------------------------------------------------------------

def _perm64():
    # evens first, then odds (within each head's 64 dims)
    return np.concatenate([np.arange(0, 64, 2), np.arange(1, 64, 2)])


def _trig_tables():
    """c32/s32: [N, 32] fp32, value of cos/sin at original dim 2i (== 2i+1)."""
    H = W = 64
    angle = 1.0 / (10000.0 ** np.linspace(0.0, 1.0, D // 4))
    angle = np.repeat(angle, 2)                          # [32]
    ih = np.arange(H, dtype=np.float64)[:, None] * angle[None, :]   # [H, 32]
    iw = np.arange(W, dtype=np.float64)[:, None] * angle[None, :]
    # full table col j<32 from sin_h (depends on row r = n//W), col 32+j from
    # sin_w (depends on col c = n%W). Original pair (2i, 2i+1):
    #   2i < 32  -> table col 2i   in h-block, base angle idx 2i
    #   2i >= 32 -> in w-block, idx 2i-32
    sin_h, cos_h = np.sin(ih), np.cos(ih)
    sin_w, cos_w = np.sin(iw), np.cos(iw)
    r = np.arange(N) // W
    c = np.arange(N) % W
    s_full = np.concatenate([sin_h[r], sin_w[c]], axis=1)   # [N, 64]
    c_full = np.concatenate([cos_h[r], cos_w[c]], axis=1)
    c32 = c_full[:, 0::2].astype(np.float32)
    s32 = s_full[:, 0::2].astype(np.float32)
    return c32, s32


def _head_masks():
    """maskS/maskN: [128, 6, 12] bf16; nonzero at (p, s, h) iff
    h == 2s + (p>=64), value SFAC resp. -1/N."""
    sfac = SCALE / N
    mS = np.zeros((128, 6, NUM_HEADS), dtype=np.float32)
    mN = np.zeros((128, 6, NUM_HEADS), dtype=np.float32)
    for p in range(128):
        for s in range(6):
            h = 2 * s + (1 if p >= 64 else 0)
            mS[p, s, h] = sfac
            mN[p, s, h] = -1.0 / N
    return mS.astype(BF16), mN.astype(BF16)


def _build_host_inputs(query, key, value, Wq, bq, Wk, bk, Wv, bv, Wo, bo):
    p64 = _perm64()
    perm = (np.arange(NUM_HEADS)[:, None] * 64 + p64[None, :]).reshape(-1)

    wq = np.ascontiguousarray(Wq.T[:, perm]).astype(BF16)
    wk = np.ascontiguousarray(Wk.T[:, perm]).astype(BF16)
    wv = np.ascontiguousarray(Wv.T).astype(BF16)
    wo = np.ascontiguousarray(Wo.T).astype(BF16)
    bqp1 = (bq[perm] + 1.0).astype(BF16)
    bkp1 = (bk[perm] + 1.0).astype(BF16)
    bvp = bv.astype(BF16)
    bof = bo.astype(np.float32)

    c32, s32 = _trig_tables()
    halves = []
    for hi in range(2):
        sl = slice(hi * NLOC, (hi + 1) * NLOC)
        ck = np.concatenate([c32[sl], c32[sl]], axis=1).astype(BF16)    # [NLOC, 64]
        s2k = np.concatenate([-s32[sl], s32[sl]], axis=1).astype(BF16)
        cq = np.tile(c32[sl].T, (4, 1)).astype(BF16)                    # [128, NLOC]
        s2q = np.tile(np.concatenate([-s32[sl].T, s32[sl].T], 0), (2, 1)).astype(BF16)
        halves.append((ck, s2k, cq, s2q))

    # one-hot G-broadcast lhsT: eblk[c][h, p] = 1 iff head h owns partition p
    # of chunk c (heads 2c: p<64, 2c+1: p>=64)
    eblk = np.zeros((6, NUM_HEADS, 128), dtype=BF16)
    for cc in range(6):
        eblk[cc, 2 * cc, :64] = 1.0
        eblk[cc, 2 * cc + 1, 64:] = 1.0

    maskS, maskN = _head_masks()

    in_maps = []
    for core in range(NCORES):
        b = core // 2
        hi = core % 2
        sl = slice(hi * NLOC, (hi + 1) * NLOC)
        ck, s2k, cq, s2q = halves[hi]
        in_maps.append({
            "xq": np.ascontiguousarray(query[b, sl].T).astype(BF16),
            "xk": np.ascontiguousarray(key[b, sl].T).astype(BF16),
            "xv": np.ascontiguousarray(value[b, sl].T).astype(BF16),
            "wq": wq, "wk": wk, "wv": wv, "wo": wo,
            "bqp1": bqp1, "bkp1": bkp1, "bv": bvp, "bo": bof,
            "cos_k": ck, "s2_k": s2k, "cos_q": cq, "s2_q": s2q,
            "eblk": eblk, "maskS": maskS, "maskN": maskN,
        })
    return in_maps


# --------------------------------------------------------------------------
# device kernel
# --------------------------------------------------------------------------

def _build_nc():
    import concourse.bacc as bacc
    import concourse.mybir as mybir
    import concourse.tile as tile

    fp32 = mybir.dt.float32
    bf16 = mybir.dt.bfloat16
    AF = mybir.ActivationFunctionType
    OP = mybir.AluOpType

    nc = bacc.Bacc("TRN2", target_bir_lowering=False, debug=False,
                   num_devices=NCORES)

    def din(name, shape, dt=bf16):
        return nc.dram_tensor(name, shape, dt, kind="ExternalInput").ap()

    xq = din("xq", [C, NLOC])
    xk = din("xk", [C, NLOC])
    xv = din("xv", [C, NLOC])
    wq_d = din("wq", [C, C])
    wk_d = din("wk", [C, C])
    wv_d = din("wv", [C, C])
    wo_d = din("wo", [C, C])
    bqp1_d = din("bqp1", [C])
    bkp1_d = din("bkp1", [C])
    bv_d = din("bv", [C])
    bo_d = din("bo", [C], fp32)
    cos_k_d = din("cos_k", [NLOC, 64])
    s2_k_d = din("s2_k", [NLOC, 64])
    cos_q_d = din("cos_q", [128, NLOC])
    s2_q_d = din("s2_q", [128, NLOC])
    eblk_d = din("eblk", [6, NUM_HEADS, 128])
    maskS_d = din("maskS", [128, 6, NUM_HEADS])
    maskN_d = din("maskN", [128, 6, NUM_HEADS])
    outT = nc.dram_tensor("outT", [C, NLOC], fp32, kind="ExternalOutput").ap()

    SFAC = SCALE / N       # kv and z scale
    NF = NLOC // 512       # 4 n-slices of 512
    NK = NLOC // 128       # 16 chunks of 128 for k/v phase

    with tile.TileContext(nc) as tc:
        from contextlib import ExitStack
        with ExitStack() as ctx:
            consts = ctx.enter_context(tc.tile_pool(name="consts", bufs=1))
            resid = ctx.enter_context(tc.tile_pool(name="resid", bufs=1))
            xin = ctx.enter_context(tc.tile_pool(name="xin", bufs=2))
            work = ctx.enter_context(tc.tile_pool(name="work", bufs=2))
            single = ctx.enter_context(tc.tile_pool(name="single", bufs=1))
            big = ctx.enter_context(tc.tile_pool(name="big", bufs=2))
            dram = ctx.enter_context(tc.tile_pool(name="dram", bufs=1, space="DRAM"))
            ph1 = ExitStack()
            pps = ph1.enter_context(tc.tile_pool(name="pps", bufs=5, space="PSUM"))
            kvp = ph1.enter_context(tc.tile_pool(name="kvp", bufs=1, space="PSUM"))

            # ---- phase-1 k-side on the sync queue (wk split so chunk-0
            # matmuls can start after half the weight transfer) ----
            bkp1_s = consts.tile([1, C], bf16, tag="bkp1")
            nc.sync.dma_start(bkp1_s[:], bkp1_d[None, :])
            bv_s = consts.tile([1, C], bf16, tag="bv")
            nc.sync.dma_start(bv_s[:], bv_d[None, :])
            wk_v = wk_d.rearrange("(s p) o -> p s o", p=128)
            wk_s = consts.tile([128, 6, C], bf16, tag="wk")
            nc.sync.dma_start(wk_s[:, 0:3, :], wk_v[:, 0:3, :])
            # (second wk half + x chunks interleave into the chunk loop)
            wv_s = consts.tile([128, 6, C], bf16, tag="wv")

            def xq_dma(nq, eng=None):
                t = xin.tile([128, 6, 512], bf16, tag="xq_t", name=f"xq{nq}")
                (eng or nc.sync).dma_start(
                    t[:], xq[:, nq * 512:(nq + 1) * 512]
                    .rearrange("(s p) n -> p s n", p=128))
                return t
            xq_tiles = {}

            # ---- v-weights + trig early on the gpsimd queue (second DMA
            # ring; xv chunk stream also rides here) ----
            wv_v = wv_d.rearrange("(s p) o -> p s o", p=128)
            nc.gpsimd.dma_start(wv_s[:, 0:3, :], wv_v[:, 0:3, :])
            cos_k_s = consts.tile([128, NK, 64], bf16, tag="cos_k")
            s2_k_s = consts.tile([128, NK, 64], bf16, tag="s2_k")
            # declared here, transferred after the chunk loop (gpsimd queue)
            wq_s = consts.tile([128, 6, C], bf16, tag="wq")
            wo_s = consts.tile([128, 6, C], bf16, tag="wo")
            bqp1_s = consts.tile([1, C], bf16, tag="bqp1")
            bo_s = consts.tile([128, 6], fp32, tag="bo")
            cos_q_s = consts.tile([128, NLOC], bf16, tag="cos_q")
            s2_q_s = consts.tile([128, NLOC], bf16, tag="s2_q")
            eblk_s = consts.tile([NUM_HEADS, 6, 128], bf16, tag="eblk")
            maskS_s = consts.tile([128, 6, NUM_HEADS], bf16, tag="maskS")
            maskN_s = consts.tile([128, 6, NUM_HEADS], bf16, tag="maskN")

            ones_row = consts.tile([1, 512], bf16, tag="ones_row")
            nc.vector.memset(ones_row[:], 1.0)
            negone = consts.tile([128, 1], fp32, tag="negone")
            nc.vector.memset(negone[:], -1.0)
            zero_col = consts.tile([1, 128], bf16, tag="zero_col")
            nc.vector.memset(zero_col[:], 0.0)

            # phase-2b staging (memset early on vector; filled post-collective)
            kv2f = single.tile([128, 6, 128], bf16, tag="kv2f")
            nc.vector.memset(kv2f[:], 0.0)
            kscat = single.tile([128, 2, 6], bf16, tag="kscat")
            nc.vector.memset(kscat[:], 0.0)
            vscat = single.tile([128, 2, 6], bf16, tag="vscat")
            nc.vector.memset(vscat[:], 0.0)

            # ---- persistent tiles ----
            qpre = big.tile([128, 6, NLOC], bf16, tag="qbig", name="qpre")
            qrope = resid.tile([128, 6, NLOC], bf16, tag="qrope")
            res = big.tile([128, 6, NLOC], bf16, tag="qbig", name="res")
            zb = resid.tile([NUM_HEADS, NLOC], bf16, tag="zb")
            kv2sb = resid.tile([128, 6, 128], bf16, tag="kv2sb")
            kblock = resid.tile([128, 6, NUM_HEADS], bf16, tag="kblock")
            wneg = resid.tile([NUM_HEADS, C], bf16, tag="wneg")

            # kv psums: 3 banks, persist through phase 1.
            # head h accumulates at [0:65, (h%4)*128 : +128] of tile h//4.
            kvps = [kvp.tile([128, 512], fp32, tag=f"kvps{t}", name=f"kvps{t}")
                    for t in range(3)]
            # start=True clears the whole bank's has_written bits, so packing 4
            # heads' accumulation groups per bank needs a single bank-wide
            # zero-write group opener; all kv matmuls then accumulate.
            for t in range(3):
                nc.tensor.matmul(kvps[t][0:65, :], zero_col[:, 0:65], ones_row[:],
                                 start=True, stop=False, skip_group_check=True)

            # ================= phase 1: k/v proj, elu, rope, kv =================
            for j in range(NK):
                xk_t = xin.tile([128, 6, 128], bf16, tag="xk_t")
                nc.sync.dma_start(
                    xk_t[:], xk[:, j * 128:(j + 1) * 128]
                    .rearrange("(s p) n -> p s n", p=128))
                xv_t = xin.tile([128, 6, 128], bf16, tag="xv_t")
                nc.gpsimd.dma_start(
                    xv_t[:], xv[:, j * 128:(j + 1) * 128]
                    .rearrange("(s p) n -> p s n", p=128))
                if j == 0:
                    nc.sync.dma_start(wk_s[:, 3:6, :], wk_v[:, 3:6, :])
                    nc.gpsimd.dma_start(wv_s[:, 3:6, :], wv_v[:, 3:6, :])
                    nc.gpsimd.dma_start(
                        cos_k_s[:], cos_k_d.rearrange("(j p) d -> p j d", p=128))
                    nc.gpsimd.dma_start(
                        s2_k_s[:], s2_k_d.rearrange("(j p) d -> p j d", p=128))
                elif j == 8:
                    xq_tiles[0] = xq_dma(0, nc.gpsimd)
                elif j == 12:
                    xq_tiles[1] = xq_dma(1, nc.gpsimd)

                vk = work.tile([128, NUM_HEADS, 128], bf16, tag="vk")
                e_t = work.tile([128, C], bf16, tag="e_t")
                kra = work.tile([128, NUM_HEADS, 66], bf16, tag="kra")
                nc.vector.memset(kra[:, :, 64:65], 1.0)

                # k projection: s-outer so each stationary x-slice is one
                # LDWEIGHTS feeding both o-halves
                psk = [pps.tile([128, 384], fp32, tag="pp384", name=f"psk{half}")
                       for half in range(2)]
                for half in range(2):
                    nc.tensor.matmul(psk[half][:], ones_row[:, 0:128],
                                     bkp1_s[:, half * 384:(half + 1) * 384],
                                     start=True, stop=False)
                for s in range(6):
                    for half in range(2):
                        nc.tensor.matmul(psk[half][:], xk_t[:, s, :],
                                         wk_s[:, s, half * 384:(half + 1) * 384],
                                         start=False, stop=(s == 5))
                for half in range(2):
                    osl = slice(half * 384, (half + 1) * 384)
                    hsl = slice(half * 6, (half + 1) * 6)
                    nc.scalar.activation(e_t[:, osl], psk[half][:], AF.Exp,
                                         bias=negone[:])
                    nc.vector.scalar_tensor_tensor(
                        vk[:, hsl, 64:128],
                        e_t[:, osl].rearrange("p (h e) -> p h e", e=64),
                        1.0, psk[half][:].rearrange("p (h e) -> p h e", e=64),
                        OP.min, OP.max)

                # v projection
                psv = [pps.tile([128, 384], fp32, tag="pp384", name=f"psv{half}")
                       for half in range(2)]
                for half in range(2):
                    nc.tensor.matmul(psv[half][:], ones_row[:, 0:128],
                                     bv_s[:, half * 384:(half + 1) * 384],
                                     start=True, stop=False)
                for s in range(6):
                    for half in range(2):
                        nc.tensor.matmul(psv[half][:], xv_t[:, s, :],
                                         wv_s[:, s, half * 384:(half + 1) * 384],
                                         start=False, stop=(s == 5))
                for half in range(2):
                    hsl = slice(half * 6, (half + 1) * 6)
                    nc.scalar.activation(
                        vk[:, hsl, 0:64],
                        psv[half][:].rearrange("p (h e) -> p h e", e=64),
                        AF.Copy)

                # rope on k_pre -> kra[:, :, 0:64]
                kpre_v = vk[:, :, 64:128]
                cosj = cos_k_s[:, j, None, :].to_broadcast([128, NUM_HEADS, 64])
                nc.vector.tensor_tensor(kra[:, :, 0:64], kpre_v, cosj, OP.mult)
                tmpb = work.tile([128, NUM_HEADS, 64], bf16, tag="tmpb")
                s2t = s2_k_s[:, j, None, 0:32].to_broadcast([128, NUM_HEADS, 32])
                s2b = s2_k_s[:, j, None, 32:64].to_broadcast([128, NUM_HEADS, 32])
                nc.vector.tensor_tensor(tmpb[:, :, 0:32], vk[:, :, 96:128], s2t, OP.mult)
                nc.vector.tensor_tensor(tmpb[:, :, 32:64], vk[:, :, 64:96], s2b, OP.mult)
                nc.vector.tensor_tensor(kra[:, :, 0:64], kra[:, :, 0:64], tmpb[:],
                                        OP.add)

                # kv accumulation: [k_rope | 1]^T @ [v | k_pre] per head
                for h in range(NUM_HEADS):
                    nc.tensor.matmul(
                        kvps[h // 4][0:65, (h % 4) * 128:(h % 4) * 128 + 128],
                        kra[:, h, 0:65], vk[:, h, :],
                        start=False, stop=(j == NK - 1), skip_group_check=True)

            # ---- q/o-side constants (gpsimd ring, after the xv stream) ----
            nc.gpsimd.dma_start(wq_s[:], wq_d.rearrange("(s p) o -> p s o", p=128))
            nc.gpsimd.dma_start(wo_s[:], wo_d.rearrange("(s p) o -> p s o", p=128))
            nc.gpsimd.dma_start(bqp1_s[:], bqp1_d[None, :])
            nc.gpsimd.dma_start(bo_s[:], bo_d.rearrange("(s p) -> p s", p=128))
            nc.gpsimd.dma_start(cos_q_s[:], cos_q_d[:])
            nc.gpsimd.dma_start(s2_q_s[:], s2_q_d[:])
            nc.gpsimd.dma_start(eblk_s[:], eblk_d.rearrange("c h p -> h c p"))
            nc.gpsimd.dma_start(maskS_s[:], maskS_d[:])
            nc.gpsimd.dma_start(maskN_s[:], maskN_d[:])

            # ================= collective: kv + sums (bf16 over the wire) ====
            kvsb = single.tile([65, 3, 512], bf16, tag="kvx", name="kvsb")
            for t in range(3):
                nc.scalar.activation(kvsb[:, t, :], kvps[t][0:65, :], AF.Copy)
            # q-proj gate: ones_g == ones_row exactly, but depends on kvsb so
            # gated q-proj slices can't start before the collective window.
            gate_f = single.tile([1, 1], fp32, tag="gate_f")
            nc.scalar.activation(gate_f[:], kvsb[0:1, 0, 0:1], AF.Copy)
            gate_0 = single.tile([1, 1], bf16, tag="gate_0")
            nc.vector.tensor_scalar_mul(gate_0[:], gate_f[:], 0.0)
            ones_g = single.tile([1, 512], bf16, tag="ones_g")
            nc.vector.tensor_tensor(ones_g[:], ones_row[:],
                                    gate_0[0:1, 0:1].to_broadcast([1, 512]),
                                    OP.add)
            bounce_in = dram.tile([65, 3 * 512], bf16, tag="b_in")
            bounce_out = dram.tile([65, 3 * 512], bf16, tag="b_out")
            nc.sync.dma_start(bounce_in[:], kvsb.rearrange("p t f -> p (t f)"))
            for nq in range(2, NF):
                xq_tiles[nq] = xq_dma(nq)
            nc.gpsimd.collective_compute(
                "AllReduce", OP.add,
                replica_groups=[[0, 1], [2, 3], [4, 5], [6, 7]],
                ins=[bounce_in.opt()], outs=[bounce_out.opt()])

            # ====== phase 2b reads: bounce -> SBUF, split across 3 queues ====
            # bounce col layout: head h spans cols [h*128, h*128+128): v-part
            # at +0..64, k-part at +64..128; partition 64 holds the n-sums.
            # cc-major and r-major views of the 1536 cols:
            bv_cc = bounce_out.rearrange("p (cc r) -> p cc r", r=256)
            bv_r = bounce_out.rearrange("p (cc r) -> p r cc", r=256)

            # kv blocks: even heads -> lower-left quads, odd -> upper-right
            # (gpsimd reads here; scalar/sync reads are emitted after the
            # phase-2a loop so their collective-wait can't block its streams)
            # post-collective gate: releases the last q-proj chunks exactly
            # when the AllReduce lands, covering the assembly latency
            gate2_sb = single.tile([1, 1], bf16, tag="gate2_sb")
            nc.gpsimd.dma_start(gate2_sb[:], bounce_out[0:1, 0:1])
            nc.gpsimd.dma_start(kv2f[0:64, :, 0:64], bv_cc[0:64, :, 0:64])
            nc.gpsimd.dma_start(kscat[0:64, 0, :], bv_r[64, 64:128, :])

            # ================= phase 2a: q proj, elu, rope =================
            # Split so the vector queue never head-of-line blocks: projections
            # for nq 0/1 and their ropes first, then the post-collective
            # assembly, then the gated nq 2/3 (whose evacuations may wait on
            # the collective-window matmuls) last.
            def q_proj(nq):
                nsl = slice(nq * 512, (nq + 1) * 512)
                xq_t = xq_tiles[nq]
                for oc in range(6):
                    if nq < 3:
                        ones_nq = ones_row
                    elif oc < 4:
                        ones_nq = ones_g
                    else:
                        ones_nq = ones_g2
                    psq = pps.tile([128, 512], fp32, tag="pp384", name="psq")
                    nc.tensor.matmul(psq[:], bqp1_s[:, oc * 128:(oc + 1) * 128],
                                     ones_nq[:], start=True, stop=False)
                    for s in range(6):
                        nc.tensor.matmul(psq[:], wq_s[:, s, oc * 128:(oc + 1) * 128],
                                         xq_t[:, s, :], start=False, stop=(s == 5))
                    e_q = work.tile([128, 512], bf16, tag="e_q")
                    nc.scalar.activation(e_q[:], psq[:], AF.Exp, bias=negone[:])
                    nc.vector.scalar_tensor_tensor(
                        qpre[:, oc, nsl], e_q[:], 1.0, psq[:], OP.min, OP.max)

            def q_rope(nq):
                nsl = slice(nq * 512, (nq + 1) * 512)
                qsw = work.tile([128, 6, 512], bf16, tag="qsw")
                for g4 in range(4):
                    sp = (g4 ^ 1) * 32
                    nc.sync.dma_start(qsw[g4 * 32:(g4 + 1) * 32, :, :],
                                      qpre[sp:sp + 32, :, nsl])
                for oc in range(6):
                    nc.vector.tensor_tensor(qrope[:, oc, nsl], qpre[:, oc, nsl],
                                            cos_q_s[:, nsl], OP.mult)
                    tmpq = work.tile([128, 512], bf16, tag="tmpq")
                    nc.vector.tensor_tensor(tmpq[:], qsw[:, oc, :], s2_q_s[:, nsl],
                                            OP.mult)
                    nc.vector.tensor_tensor(qrope[:, oc, nsl], qrope[:, oc, nsl],
                                            tmpq[:], OP.add)

            q_proj(0)
            q_proj(1)
            q_rope(0)
            q_rope(1)
            q_proj(2)

            # remaining bounce reads (sync queue is clear of waits by now)
            nc.sync.dma_start(kv2f[64:128, :, 64:128], bv_cc[0:64, :, 128:192])
            nc.sync.dma_start(kscat[64:128, 1, :], bv_r[64, 192:256, :])
            nc.sync.dma_start(vscat[0:64, 0, :], bv_r[64, 0:64, :])
            nc.sync.dma_start(vscat[64:128, 1, :], bv_r[64, 128:192, :])

            # kv2sb scale on DVE (between ropes; waits only on the kv2f reads)
            gate2_0 = single.tile([1, 1], bf16, tag="gate2_0")
            nc.vector.tensor_scalar_mul(gate2_0[:], gate2_sb[:], 0.0)
            ones_g2 = single.tile([1, 512], bf16, tag="ones_g2")
            nc.vector.tensor_tensor(ones_g2[:], ones_row[:],
                                    gate2_0[0:1, 0:1].to_broadcast([1, 512]),
                                    OP.add)
            nc.vector.tensor_scalar_mul(kv2sb[:], kv2f[:], SFAC)
            # kblock/vbneg mask-multiplies on gpsimd (idle post-collective)
            # [128, 12] head-ordered views as [128, hh, par] (h = 2*hh + par)
            kscat_v = kscat.rearrange("p a b -> p b a")
            vscat_v = vscat.rearrange("p a b -> p b a")
            vbneg = single.tile([128, 6, NUM_HEADS], bf16, tag="vbneg")
            for s in range(6):
                nc.vector.tensor_tensor(
                    kblock[:, s, :].rearrange("p (b a) -> p b a", a=2),
                    kscat_v,
                    maskS_s[:, s, :].rearrange("p (b a) -> p b a", a=2),
                    OP.mult)
                nc.vector.tensor_tensor(
                    vbneg[:, s, :].rearrange("p (b a) -> p b a", a=2),
                    vscat_v,
                    maskN_s[:, s, :].rearrange("p (b a) -> p b a", a=2),
                    OP.mult)

            q_proj(3)

            ph1.close()
            pz = ctx.enter_context(tc.tile_pool(name="pz", bufs=2, space="PSUM"))
            pg = ctx.enter_context(tc.tile_pool(name="pg", bufs=2, space="PSUM"))
            pa = ctx.enter_context(tc.tile_pool(name="pa", bufs=2, space="PSUM"))
            po = ctx.enter_context(tc.tile_pool(name="po", bufs=2, space="PSUM"))

            # wneg = -(W~)^T : [12, 768]  (PE work; emitted after phase 2a so
            # its wait on the collective can't block q-proj matmuls)
            for half in range(2):
                osl = slice(half * 384, (half + 1) * 384)
                psw = pg.tile([128, 512], fp32, tag="psg", name="psw")[0:NUM_HEADS, 0:384]
                for s in range(6):
                    nc.tensor.matmul(psw[:], vbneg[:, s, :], wo_s[:, s, osl],
                                     start=(s == 0), stop=(s == 5))
                nc.scalar.activation(wneg[:, osl], psw[:], AF.Copy)

            # == phase 2c+3+4: per-n-slice chain z -> g -> q@kv -> o_proj
            def chain(nq):
                nsl = slice(nq * 512, (nq + 1) * 512)
                psz = pz.tile([128, 512], fp32, tag="psz", name="psz")[0:NUM_HEADS, :]
                for s in range(6):
                    nc.tensor.matmul(psz[:], kblock[:, s, :], qpre[:, s, nsl],
                                     start=(s == 0), stop=(s == 5))
                gf = work.tile([NUM_HEADS, 512], fp32, tag="gf")
                nc.vector.reciprocal_approx_fast(gf[:], psz[:])  # z >= 7, eps moot
                gb = work.tile([NUM_HEADS, 512], bf16, tag="gb")
                nc.vector.tensor_scalar_add(gb[:], gf[:], 1.0)
                nc.vector.tensor_copy(zb[:, nsl], psz[:])
                for cc in range(6):
                    psg = pg.tile([128, 512], fp32, tag="psg", name="psg")
                    nc.tensor.matmul(psg[:], eblk_s[:, cc, :], gb[:],
                                     start=True, stop=True)
                    gx = work.tile([128, 512], bf16, tag="gx")
                    nc.scalar.activation(gx[:], psg[:], AF.Copy)
                    psa = pa.tile([128, 512], fp32, tag="psa", name="psa")
                    nc.tensor.matmul(psa[:], kv2sb[:, cc, :], qrope[:, cc, nsl],
                                     start=True, stop=True)
                    # res = (q@kv) * (1 + 1/z); psa is independent of the
                    # g-chain so the PE never waits on the reciprocal.
                    nc.vector.tensor_tensor(res[:, cc, nsl], psa[:], gx[:],
                                            OP.mult)
                # o_proj for this n-slice (fills PE while next chain's DVE runs)
                for c2 in range(6):
                    c2sl = slice(c2 * 128, (c2 + 1) * 128)
                    pso = po.tile([128, 512], fp32, tag="pso", name="pso")
                    nc.tensor.matmul(pso[:], wneg[:, c2sl], zb[:, nsl],
                                     start=True, stop=False)
                    for s in range(6):
                        nc.tensor.matmul(pso[:], wo_s[:, s, c2sl], res[:, s, nsl],
                                         start=False, stop=(s == 5))
                    osb = work.tile([128, 512], fp32, tag="osb")
                    if c2 % 2 == 0:
                        nc.scalar.activation(osb[:], pso[:], AF.Identity,
                                             bias=bo_s[:, c2:c2 + 1])
                    else:
                        nc.vector.tensor_tensor(
                            osb[:], pso[:],
                            bo_s[:, c2:c2 + 1].to_broadcast([128, 512]),
                            OP.add)
                    (nc.gpsimd if c2 % 2 == 0 else nc.sync).dma_start(
                        outT[c2sl, nsl], osb[:])

            chain(0)
            q_rope(2)
            q_rope(3)
            chain(1)
            chain(2)
            chain(3)

    nc.compile()
    return nc


def _get_nc():
    if "nc" not in _CACHE:
        _CACHE["nc"] = _build_nc()
    return _CACHE["nc"]


# --------------------------------------------------------------------------
# entry point
# --------------------------------------------------------------------------

def kernel(query, key, value, Wq, bq, Wk, bk, Wv, bv, Wo, bo, H, W):
    from concourse.bass_utils import run_bass_kernel_spmd

    assert int(H) == 64 and int(W) == 64
    query = np.asarray(query, np.float32)
    key = np.asarray(key, np.float32)
    value = np.asarray(value, np.float32)
    in_maps = _build_host_inputs(
        query, key, value,
        np.asarray(Wq, np.float32), np.asarray(bq, np.float32),
        np.asarray(Wk, np.float32), np.asarray(bk, np.float32),
        np.asarray(Wv, np.float32), np.asarray(bv, np.float32),
        np.asarray(Wo, np.float32), np.asarray(bo, np.float32))

    nc = _get_nc()
    kwargs = {}
    if os.environ.get("KERNEL_TRACE") == "1":
        kwargs = dict(trace=True, tmpdir=tempfile.mkdtemp(prefix="malat_"))
    r = run_bass_kernel_spmd(nc, in_maps, core_ids=list(range(NCORES)), **kwargs)
    LAST_RESULTS[0] = r

    out = np.empty((B, N, C), np.float32)
    for core in range(NCORES):
        b = core // 2
        sl = slice((core % 2) * NLOC, (core % 2 + 1) * NLOC)
        out[b, sl, :] = r.results[core]["outT"].T
    return out
